# revision 1
# baseline (speedup 1.0000x reference)
"""Trainium2 Bass/Tile kernel for an RNN-T Joiner:

    enc_p = encoder_out @ W_enc.T + b_enc          (N,200,512)
    dec_p = decoder_out @ W_dec.T + b_dec          (N,50,512)
    act   = tanh(enc_p[:,:,None,:] + dec_p[:,None,:,:])
    out   = act @ W_out.T + b_out                  (N,200,50,500)

Sharding: data-parallel over N=8 — core i computes batch element i end to
end; the small weight matrices are replicated to every core.

Per-core dataflow (all on-chip after the initial loads):
  - load enc/dec/W_enc/W_dec/W_out in natural layout, PE-transpose the
    128x128 blocks so every contraction operand has its contraction dim on
    the partition axis,
  - project:  enc_pT[j,t], dec_pT[j,u]  (PE, fp32, biases folded in via the
    ACT copy from PSUM),
  - broadcast-add (DVE, 0-stride APs) + in-place tanh (ACT) to build
    actT[j, cell] for 64-wide t-chunks (cell = t*50+u),
  - vocab matmul per 128-cell block: psum[cell,v] = sum_jb actT_blk.T @ W_outT
    (float32r: full-rate fp32 on the PE for moving dim >= 256),
  - +b_out fused into the PSUM->SBUF copy (DVE tensor_tensor with a
    pre-broadcast bias tile), output DMA in ~1.25MB batches.
"""

import numpy as np
from contextlib import ExitStack

N, T, U = 8, 200, 50
E = J = 512
V = 500
CELLS = T * U
P = 128
KB = J // P  # 4 contraction blocks

_NC_CACHE = {}


def _build_nc():
    import concourse.mybir as mybir
    import concourse.tile as tile
    from concourse import bacc
    from concourse.masks import make_identity

    f32 = mybir.dt.float32
    f32r = mybir.dt.float32r
    ADD = mybir.AluOpType.add
    TANH = mybir.ActivationFunctionType.Tanh
    IDENT = mybir.ActivationFunctionType.Identity

    nc = bacc.Bacc("TRN2", target_bir_lowering=False, debug=False)

    enc_d = nc.dram_tensor("encoder_out", [T, E], f32, kind="ExternalInput").ap()
    dec_d = nc.dram_tensor("decoder_out", [U, E], f32, kind="ExternalInput").ap()
    wenc_d = nc.dram_tensor("W_enc", [J, E], f32, kind="ExternalInput").ap()
    benc_d = nc.dram_tensor("b_enc", [J], f32, kind="ExternalInput").ap()
    wdec_d = nc.dram_tensor("W_dec", [J, E], f32, kind="ExternalInput").ap()
    bdec_d = nc.dram_tensor("b_dec", [J], f32, kind="ExternalInput").ap()
    wout_d = nc.dram_tensor("W_out", [V, J], f32, kind="ExternalInput").ap()
    bout_d = nc.dram_tensor("b_out", [V], f32, kind="ExternalInput").ap()
    out_d = nc.dram_tensor("logits", [CELLS, V], f32, kind="ExternalOutput").ap()

    with tile.TileContext(nc) as tc, ExitStack() as ctx:
        const = ctx.enter_context(tc.tile_pool(name="const", bufs=1))
        stage = ctx.enter_context(tc.tile_pool(name="stage", bufs=2))
        tp_ps = ctx.enter_context(tc.tile_pool(name="tp_ps", bufs=2, space="PSUM"))
        pj_ps = ctx.enter_context(tc.tile_pool(name="pj_ps", bufs=2, space="PSUM"))
        mm_ps = ctx.enter_context(tc.tile_pool(name="mm_ps", bufs=4, space="PSUM"))
        act_pool = ctx.enter_context(tc.tile_pool(name="act", bufs=2))
        out_pool = ctx.enter_context(tc.tile_pool(name="outp", bufs=3))

        ident = const.tile([P, P], f32)
        make_identity(nc, ident)

        # Per-partition bias columns: b[kb*128+p] -> [p, kb]
        b_enc_sb = const.tile([P, KB], f32)
        nc.sync.dma_start(b_enc_sb[:], benc_d.rearrange("(kb p) -> p kb", p=P))
        b_dec_sb = const.tile([P, KB], f32)
        nc.sync.dma_start(b_dec_sb[:], bdec_d.rearrange("(kb p) -> p kb", p=P))

        # b_out broadcast to all 128 partitions via a K=1 ones matmul
        bout_row = const.tile([1, V], f32)
        nc.sync.dma_start(bout_row[:], bout_d[None, :])
        ones_col = const.tile([1, P], f32)
        nc.gpsimd.memset(ones_col[:], 1.0)
        bp = mm_ps.tile([P, V], f32, tag="mm")
        nc.tensor.matmul(bp[:], lhsT=ones_col[:], rhs=bout_row[:], start=True, stop=True)
        bout_rep = const.tile([P, V], f32)
        nc.vector.tensor_copy(bout_rep[:], bp[:])

        def load_transposed(dram_ap, rows, name, dtype=f32):
            """dram [rows, 512] natural -> KB tiles [128, rows] with the
            512-dim on partitions (PE block transpose + ACT copy)."""
            tiles = [const.tile([P, rows], dtype, name=f"{name}T{kb}") for kb in range(KB)]
            n_rt = (rows + P - 1) // P
            for rt in range(n_rt):
                r0 = rt * P
                rsz = min(P, rows - r0)
                nat = stage.tile([P, E], f32, tag="stage", name=f"{name}_nat{rt}")
                nc.sync.dma_start(nat[:rsz, :], dram_ap[r0 : r0 + rsz, :])
                for kb in range(KB):
                    tp = tp_ps.tile([P, P], f32, tag="tp", name=f"{name}_tp{rt}_{kb}")
                    nc.tensor.transpose(
                        tp[:, :rsz], nat[:rsz, kb * P : (kb + 1) * P], ident[:rsz, :rsz]
                    )
                    nc.scalar.copy(tiles[kb][:, r0 : r0 + rsz], tp[:, :rsz])
            return tiles

        # Emission order = scheduler priority: the projections (and hence the
        # first chunk's act generation) gate everything, so load their
        # operands first; W_outT is only needed by the first vocab matmul.
        encT = load_transposed(enc_d, T, "enc")      # 4 x [128(e), 200(t)]
        decT = load_transposed(dec_d, U, "dec")      # 4 x [128(e), 50(u)]
        W_encT = load_transposed(wenc_d, J, "wenc")  # 4 x [128(e), 512(j)]
        W_decT = load_transposed(wdec_d, J, "wdec")  # 4 x [128(e), 512(j)]

        # Projections -> enc_pT[jb]: [128(j), T], dec_pT[jb]: [128(j), U]
        def project(WT, srcT, b_sb, width, nm):
            outs = []
            for jb in range(KB):
                pp = pj_ps.tile([P, T], f32, tag="pj", name=f"{nm}_ps{jb}")
                for kb in range(KB):
                    nc.tensor.matmul(
                        pp[:, :width],
                        lhsT=WT[kb][:, jb * P : (jb + 1) * P],
                        rhs=srcT[kb][:, :width],
                        start=(kb == 0),
                        stop=(kb == KB - 1),
                    )
                o = const.tile([P, width], f32, name=f"{nm}{jb}")
                nc.scalar.activation(o[:], pp[:, :width], IDENT, bias=b_sb[:, jb : jb + 1])
                outs.append(o)
            return outs

        enc_pT = project(W_encT, encT, b_enc_sb, T, "encp")
        dec_pT = project(W_decT, decT, b_dec_sb, U, "decp")

        W_outT = load_transposed(wout_d, V, "wout", dtype=f32r)  # 4 x [128(j), 500(v)]

        # Main loop: cell = t*U+u, t-chunks of 64 (64*50 = 3200 = 25*128)
        CHUNKS = [(0, 64), (64, 64), (128, 64), (192, 8)]
        ACT_COLS = 64 * U
        BATCH = 5  # output blocks per DMA (5*128 cells * 2000B = 1.28 MB)
        wout_r = [W_outT[jb][:, :V] for jb in range(KB)]

        for ci, (t0, L) in enumerate(CHUNKS):
            C = L * U
            c_base = t0 * U
            acts = []
            for jb in range(KB):
                s = act_pool.tile([P, ACT_COLS], f32r, tag=f"act{jb}", name=f"s{ci}_{jb}")
                # DVE is the busiest non-PE engine (it also drains every
                # vocab-matmul PSUM); route half the broadcast-adds to the
                # otherwise-idle GPSIMD to balance.
                add_eng = nc.vector if jb % 2 == 0 else nc.gpsimd
                add_eng.tensor_tensor(
                    out=s[:, :C].rearrange("p (l u) -> p l u", u=U),
                    in0=dec_pT[jb][:, None, :].broadcast_to([P, L, U]),
                    in1=enc_pT[jb][:, t0 : t0 + L][:, :, None].broadcast_to([P, L, U]),
                    op=ADD,
                )
                # tanh in halves so the first blocks' matmuls can start
                # before the whole chunk is activated
                h = C // 2
                nc.scalar.activation(s[:, :h], s[:, :h].bitcast(f32), TANH)
                nc.scalar.activation(s[:, h:C], s[:, h:C].bitcast(f32), TANH)
                acts.append(s)

            nfull = C // P
            tail = C % P
            b0 = 0
            while b0 < nfull:
                nb = min(BATCH, nfull - b0)
                ob = out_pool.tile([P, BATCH * V], f32, tag="ob", name=f"ob{ci}_{b0}")
                for q in range(nb):
                    blk = b0 + q
                    ps = mm_ps.tile([P, V], f32, tag="mm", name=f"ps{ci}_{blk}")
                    for jb in range(KB):
                        nc.tensor.matmul(
                            ps[:],
                            lhsT=acts[jb][:, blk * P : (blk + 1) * P],
                            rhs=wout_r[jb],
                            start=(jb == 0),
                            stop=(jb == KB - 1),
                        )
                    nc.vector.tensor_tensor(
                        out=ob[:, q * V : (q + 1) * V], in0=ps[:], in1=bout_rep[:], op=ADD
                    )
                c0 = c_base + b0 * P
                dst = out_d[c0 : c0 + nb * P, :].rearrange("(b p) v -> p b v", p=P)
                nc.sync.dma_start(dst, ob[:, : nb * V].rearrange("p (b v) -> p b v", v=V))
                b0 += nb
            if tail:
                ps = mm_ps.tile([P, V], f32, tag="mm", name=f"ps{ci}_t")
                for jb in range(KB):
                    nc.tensor.matmul(
                        ps[:tail, :],
                        lhsT=acts[jb][:, nfull * P : nfull * P + tail],
                        rhs=wout_r[jb],
                        start=(jb == 0),
                        stop=(jb == KB - 1),
                    )
                obt = out_pool.tile([P, BATCH * V], f32, tag="ob", name=f"obt{ci}")
                nc.vector.tensor_tensor(
                    out=obt[:tail, :V], in0=ps[:tail, :], in1=bout_rep[:tail, :], op=ADD
                )
                c0 = c_base + nfull * P
                nc.sync.dma_start(out_d[c0 : c0 + tail, :], obt[:tail, :V])

    nc.compile()
    return nc


def get_nc():
    if "nc" not in _NC_CACHE:
        _NC_CACHE["nc"] = _build_nc()
    return _NC_CACHE["nc"]


def make_in_maps(inputs):
    enc = np.ascontiguousarray(np.asarray(inputs["encoder_out"], dtype=np.float32))
    dec = np.ascontiguousarray(np.asarray(inputs["decoder_out"], dtype=np.float32))
    shared = {
        k: np.ascontiguousarray(np.asarray(inputs[k], dtype=np.float32))
        for k in ("W_enc", "b_enc", "W_dec", "b_dec", "W_out", "b_out")
    }
    return [
        {"encoder_out": enc[i], "decoder_out": dec[i], **shared} for i in range(N)
    ]


def kernel(**inputs):
    from concourse.bass_utils import run_bass_kernel_spmd

    nc = get_nc()
    in_maps = make_in_maps(inputs)
    res = run_bass_kernel_spmd(nc, in_maps, core_ids=list(range(N)))
    out = np.stack([r["logits"] for r in res.results], axis=0)
    return out.reshape(N, T, U, V)

